# revision 23
# baseline (speedup 1.0000x reference)
"""Batched multi-head graph attention (GAT) kernel for 8 Trainium2 NeuronCores.

Math (per batch b, head h):
    hp      = h[b] @ w[h]                          # [N, F]
    t       = tanh(hp)
    s       = t @ a_src[h];  d = t @ a_dst[h]      # [N]
    score   = leaky_relu(s_i + d_j, 0.2)
    e       = where(adj>0, exp(score), 0)
    out     = (e / e.sum(-1, keepdim)) @ hp + bias

On-device identity (v2-folded form):
    exp(leaky(z)) = e^{0.2 s_i} * v2_j * max(q_i w_j, 1)
    with q = e^{0.8 s}, w = e^{0.8 d}, v2 = e^{0.2 d}.  The e^{0.2 s_i}
    row factor cancels in softmax.  The v2_j column factor is folded into
    the PE stationary [v2*hp | v2], so the per-element work is ONE
    tensor_scalar (4x DVE mode):  ea = max(q_i * w_j, 1)
    then ONE mask multiply (2x DVE):  da = ea * adjT
    and a PE matmul accumulating numerator and denominator together.

    ACT-offload variant for some (h,jb) blocks: ea_act = relu(q w - 1) on
    the Scalar engine; the missing "+1" is restored by an extra matmul of
    the SAME stationary [v2*hp | v2] against the raw adjT mask (exact).

adj mask trick: adj values 0.0/1.0 fp32; the high uint16 halves bitcast to
fp16 {0, 1.875} -- a constant scale on every surviving term that cancels
in the normalization.  Host passes adj TRANSPOSED as uint16 high halves.

Sharding: 8 cores = 4 batches x 2 head-pairs; each core handles 2 heads for
ALL 2048 query rows against all 2048 keys.  Output is fp16 (host upcasts);
the PSUM spill is scaled by 2^-6 which cancels in the num/den ratio.
"""

import os
from contextlib import ExitStack

import numpy as np

import concourse.bass as bass
import concourse.mybir as mybir
import concourse.tile as tile
from concourse import bacc
from concourse.bass_utils import run_bass_kernel_spmd
from concourse.masks import make_identity

F32 = mybir.dt.float32
F16 = mybir.dt.float16
U16 = mybir.dt.uint16
ALU = mybir.AluOpType
ACTF = mybir.ActivationFunctionType
AX = mybir.AxisListType

B, N, H, F = 4, 2048, 4, 64
NCORES = 8
ROWS = N               # query rows per core (full)
KEYS = N               # keys per core (full)
HEADS_PER = 2          # heads per core
NEG_SLOPE = 0.2
SPILL_SCALE = 2.0 ** -6


def default_assign(jb, h):
    """ea engine for block (jb, head h): "act" -> Scalar relu + PE fixup,
    "dve" -> Vector tensor_scalar max."""
    return "act" if (jb + h) % 2 == 0 else "dve"


# da blocks routed to the (otherwise idle) Pool engine; chosen among
# act-path blocks so those (jb,h) blocks bypass the Vector engine entirely
DEFAULT_POOL_DA = frozenset({(1, 1), (5, 1), (9, 1), (13, 1)})


def build_program(rows=ROWS, keys=KEYS, heads=HEADS_PER, f=F,
                  assign=default_assign, pool_da=DEFAULT_POOL_DA):
    nc = bacc.Bacc("TRN2", target_bir_lowering=False, debug=False)

    kb = keys // 128           # key blocks
    qb = rows // 128           # query blocks (for q transposes)
    nhalf = rows // 512        # 512-wide output column chunks per head
    fe = f + 1                 # [v2*hp | v2] stationary width

    hbT_d = nc.dram_tensor("hbT", [f, keys], F32, kind="ExternalInput")
    adjt_d = nc.dram_tensor("adjt", [keys, rows], U16, kind="ExternalInput")
    w_d = nc.dram_tensor("wmat", [heads, f, f], F32, kind="ExternalInput")
    # host pre-broadcasts the attention vectors to all 128 partitions
    ap_d = nc.dram_tensor("apairb", [128, heads, 2, f], F32,
                          kind="ExternalInput")
    out_d = nc.dram_tensor("out", [heads, rows, f], F16,
                           kind="ExternalOutput")

    with tile.TileContext(nc) as tc:
        with (
            tc.tile_pool(name="const", bufs=1) as const,
            tc.tile_pool(name="persist", bufs=1) as persist,
            tc.tile_pool(name="stmp", bufs=2) as stmp,
        ):
            id16 = const.tile([128, 128], F16, tag="id16")
            make_identity(nc, id16)
            neg1 = const.tile([128, 1], F32, tag="neg1")
            nc.vector.memset(neg1, -1.0)

            # ---- global loads (small unblockers first) ------------------
            abc32 = persist.tile([128, heads, 2, f], F32, tag="abc32")
            nc.sync.dma_start(out=abc32, in_=ap_d.ap())
            hT32 = persist.tile([f, keys], F32, tag="hT32")
            nc.sync.dma_start(out=hT32, in_=hbT_d.ap())
            w32 = persist.tile([f, heads, f], F32, tag="w32")
            nc.sync.dma_start(out=w32, in_=w_d.ap().rearrange("h f o -> f h o"))

            a16 = persist.tile([128, heads, 2, f], F16, tag="a16")
            nc.vector.tensor_copy(a16, abc32)
            w16 = persist.tile([f, heads, f], F16, tag="w16")
            nc.vector.tensor_copy(w16, w32)
            hT16 = persist.tile([f, keys], F16, tag="hT16")
            # cast in halves so the first hp matmul group starts earlier
            nc.scalar.activation(hT16[:, 0:keys // 2], hT32[:, 0:keys // 2],
                                 ACTF.Copy)
            nc.scalar.activation(hT16[:, keys // 2:], hT32[:, keys // 2:],
                                 ACTF.Copy)

            # ---- prefetch all transposed mask blocks (streamed pool) ----
            # (declared early so DMA runs under the whole setup phase)
            adjp_stack = ExitStack()
            adjp = adjp_stack.enter_context(
                tc.tile_pool(name="adjp", bufs=8))
            adjts = []
            for jb in range(kb):
                adjt_t = adjp.tile([128, rows], U16, tag="adjt",
                                   name=f"adjt{jb}")
                nc.sync.dma_start(
                    out=adjt_t, in_=adjt_d.ap()[jb * 128:(jb + 1) * 128, :])
                adjts.append(adjt_t)

            # ---- per-head setup (head 0's chain completes first) --------
            tanh16 = persist.tile([128, heads, kb, f], F16, tag="tanh16")
            hpt2 = []    # [128, kb, fe] f16 -- [v2*hp | v2] stationaries
            wcol = []    # [128, kb] f32 -- e^{0.8 d}
            qbc = []     # [128, rows] f16 -- e^{0.8 s} broadcast
            with (
                tc.tile_pool(name="psum_hp", bufs=2, space="PSUM") as php,
                tc.tile_pool(name="psum_q", bufs=2, space="PSUM") as pq,
            ):
                for h in range(heads):
                    hpt_h = stmp.tile([128, kb, fe], F16, name=f"hptmp{h}",
                                      tag=f"hptmp{h}")
                    g_hp = min(8, kb)
                    for g in range(kb // g_hp):
                        pp = php.tile([128, g_hp * f], F32, tag="php")
                        for t in range(g_hp):
                            blk = g * g_hp + t
                            nc.tensor.matmul(
                                pp[:, t * f:(t + 1) * f],
                                lhsT=hT16[:, blk * 128:(blk + 1) * 128],
                                rhs=w16[:, h, :], start=True, stop=True)
                        nc.scalar.activation(
                            tanh16[:, h, g * g_hp:(g + 1) * g_hp, :],
                            pp.rearrange("p (t o) -> p t o", o=f),
                            ACTF.Tanh)
                        nc.scalar.activation(
                            hpt_h[:, g * g_hp:(g + 1) * g_hp, 0:f],
                            pp.rearrange("p (t o) -> p t o", o=f),
                            ACTF.Identity)
                    nc.vector.memset(hpt_h[:, :, f:fe], 1.0)

                    # s, d for this head: prod + blockwise reduce (s first:
                    # it unblocks the q chain before d finishes)
                    prod = stmp.tile([128, kb, 2, f], F16, name=f"prod{h}",
                                     tag=f"prod{h}")
                    nc.vector.tensor_tensor(
                        out=prod,
                        in0=tanh16[:, h].unsqueeze(2).broadcast_to(
                            [128, kb, 2, f]),
                        in1=a16[:, h].unsqueeze(1).broadcast_to(
                            [128, kb, 2, f]),
                        op=ALU.mult)
                    sums = stmp.tile([128, kb, 2], F32, name=f"sums{h}",
                                     tag=f"sums{h}")
                    nc.vector.reduce_sum(sums[:, :, 0:1], prod[:, :, 0:1, :],
                                         axis=AX.X)

                    # q chain: exp col (f16), transpose to row, evac,
                    # broadcast (gpsimd runs while DVE does the d reduce)
                    qc16 = stmp.tile([128, kb], F16, name=f"qc{h}",
                                     tag=f"qc{h}")
                    nc.scalar.activation(qc16, sums[:, :, 0], ACTF.Exp,
                                         scale=1.0 - NEG_SLOPE)
                    pq_t = pq.tile([1, rows], F16, tag="pq")
                    for t in range(qb):
                        nc.tensor.transpose(
                            pq_t[:, t * 128:(t + 1) * 128],
                            qc16[:, t:t + 1], id16)
                    qrow = stmp.tile([1, rows], F16, name=f"qrow{h}",
                                     tag=f"qrow{h}")
                    nc.vector.tensor_copy(qrow, pq_t)
                    qb_h = persist.tile([128, rows], F16, tag=f"qb{h}")
                    nc.gpsimd.partition_broadcast(qb_h, qrow)
                    qbc.append(qb_h)

                    # d reduce + w = e^{0.8 d} (scalar for ea), v2 = e^{0.2
                    # d} (f16, folded into the stationary)
                    nc.vector.reduce_sum(sums[:, :, 1:2], prod[:, :, 1:2, :],
                                         axis=AX.X)
                    w_h = persist.tile([128, kb], F32, tag=f"w{h}")
                    nc.scalar.activation(w_h, sums[:, :, 1], ACTF.Exp,
                                         scale=1.0 - NEG_SLOPE)
                    wcol.append(w_h)
                    v2c = stmp.tile([128, kb], F16, name=f"v2c{h}",
                                    tag=f"v2c{h}")
                    nc.scalar.activation(v2c, sums[:, :, 1], ACTF.Exp,
                                         scale=NEG_SLOPE)
                    hpt2_h = persist.tile([128, kb, fe], F16, tag=f"hpt2{h}")
                    nc.gpsimd.tensor_tensor(
                        out=hpt2_h, in0=hpt_h,
                        in1=v2c.unsqueeze(2).broadcast_to([128, kb, fe]),
                        op=ALU.mult)
                    hpt2.append(hpt2_h)

            # ---- main loop: masked weights + fused matmul ---------------
            nacc = heads * nhalf
            acc_sb = persist.tile([fe, nacc, 512], F16, tag="acc_sb")
            accp_stack = ExitStack()
            accp = accp_stack.enter_context(
                tc.tile_pool(name="accp", bufs=1, space="PSUM"))
            accs = {}
            for h in range(heads):
                for half in range(nhalf):
                    i = h * nhalf + half
                    accs[i] = accp.tile([fe, 512], F32, tag=f"acc{i}",
                                        name=f"acc{i}")

            with (
                tc.tile_pool(name="ep", bufs=4) as ep,
                tc.tile_pool(name="dp", bufs=4) as dp,
            ):
                for jb in range(kb):
                    adj16 = adjts[jb].bitcast(F16)
                    last = jb == kb - 1
                    for h in range(heads):
                        is_act = assign(jb, h) == "act"
                        w_s = wcol[h][:, jb:jb + 1]
                        ea = ep.tile([128, rows], F16, tag=f"ea{h}",
                                     name=f"ea{h}_{jb}")
                        if is_act:
                            nc.scalar.activation(
                                ea, qbc[h], ACTF.Relu,
                                bias=neg1, scale=w_s)
                        else:
                            nc.vector.tensor_scalar(
                                out=ea, in0=qbc[h],
                                scalar1=w_s, scalar2=1.0,
                                op0=ALU.mult, op1=ALU.max)
                        da = dp.tile([128, rows], F16, tag=f"da{h}",
                                     name=f"da{h}_{jb}")
                        da_eng = (nc.gpsimd if (jb, h) in pool_da
                                  else nc.vector)
                        da_eng.tensor_tensor(out=da, in0=ea, in1=adj16,
                                             op=ALU.mult)
                        for half in range(nhalf):
                            sl = slice(half * 512, (half + 1) * 512)
                            nc.tensor.matmul(
                                accs[h * nhalf + half],
                                lhsT=hpt2[h][:, jb, :],
                                rhs=da[:, sl],
                                start=(jb == 0),
                                stop=(last and not is_act))
                            if is_act:
                                nc.tensor.matmul(
                                    accs[h * nhalf + half],
                                    lhsT=hpt2[h][:, jb, :],
                                    rhs=adj16[:, sl],
                                    start=False, stop=last)

                # spill accumulators (scaled; scale cancels in num/den);
                # alternate engines so the copies drain in parallel
                for i in range(nacc):
                    if i % 2 == 0:
                        nc.scalar.activation(acc_sb[:, i, :], accs[i],
                                             ACTF.Identity,
                                             scale=SPILL_SCALE)
                    else:
                        nc.vector.tensor_scalar_mul(acc_sb[:, i, :],
                                                    accs[i], SPILL_SCALE)
            accp_stack.close()
            adjp_stack.close()

            # ---- normalize + store (fp16) -------------------------------
            nq = 512 // 128
            with (
                tc.tile_pool(name="ptf", bufs=2, space="PSUM") as ptf,
                tc.tile_pool(name="outp", bufs=4) as outp,
            ):
                for h in range(heads):
                    for half in range(nhalf):
                        i = h * nhalf + half
                        pt = ptf.tile([128, nq, fe + 1], F16, tag="pt")
                        for q in range(nq):
                            nc.tensor.transpose(
                                pt[:, q, 0:fe],
                                acc_sb[:, i, q * 128:(q + 1) * 128],
                                id16[0:fe, 0:fe])
                        rcol = outp.tile([128, nq], F32, tag="rcol")
                        nc.vector.reciprocal(rcol, pt[:, :, f])
                        rc16 = outp.tile([128, nq], F16, tag="rc16")
                        nc.vector.tensor_copy(rc16, rcol)
                        osb = outp.tile([128, nq, f], F16, tag="osb")
                        nc.vector.tensor_tensor(
                            out=osb, in0=pt[:, :, 0:f],
                            in1=rc16.unsqueeze(2).broadcast_to(
                                [128, nq, f]),
                            op=ALU.mult)
                        nc.sync.dma_start(
                            out=out_d.ap()[
                                h, half * 512:(half + 1) * 512, :]
                            .rearrange("(q p) f -> p q f", p=128),
                            in_=osb)
    nc.compile()
    return nc


_PROGRAM_CACHE = {}


def _get_program():
    key = "full"
    if key not in _PROGRAM_CACHE:
        _PROGRAM_CACHE[key] = build_program()
    return _PROGRAM_CACHE[key]


def make_in_maps(h, adj, w, a_src, a_dst):
    """Shard + marshal the full inputs into 8 per-core input maps."""
    h = np.ascontiguousarray(np.asarray(h, dtype=np.float32))
    adj = np.ascontiguousarray(np.asarray(adj, dtype=np.float32))
    w = np.ascontiguousarray(np.asarray(w, dtype=np.float32))
    apairt_all = np.concatenate(
        [np.asarray(a_src)[:, None, :, 0], np.asarray(a_dst)[:, None, :, 0]],
        axis=1).astype(np.float32)  # [H, 2, F]
    in_maps = []
    hbT = {}
    adjt = {}
    for b in range(B):
        hbT[b] = np.ascontiguousarray(h[b].T)  # [F, N]
        adjT = np.ascontiguousarray(adj[b].T)  # [N, N] keys x queries
        adjt[b] = np.ascontiguousarray(
            adjT.view(np.uint16).reshape(N, N, 2)[:, :, 1])
    for c in range(NCORES):
        b, p = c // 2, c % 2
        apair = apairt_all[2 * p:2 * p + 2]  # [2, 2, F]
        apairb = np.ascontiguousarray(
            np.broadcast_to(apair[None], (128,) + apair.shape))
        in_maps.append({
            "hbT": hbT[b],
            "adjt": adjt[b],
            "wmat": np.ascontiguousarray(w[2 * p:2 * p + 2]),
            "apairb": apairb,
        })
    return in_maps


def assemble_output(results, bias):
    """Gather per-core [2, N, F] fp16 results into [B, H, N, F] fp32."""
    out = np.empty((B, H, N, F), dtype=np.float32)
    for c in range(NCORES):
        b, p = c // 2, c % 2
        out[b, 2 * p:2 * p + 2, :, :] = results[c]["out"].astype(np.float32)
    if bias is not None:
        out = out + np.asarray(bias, dtype=np.float32)[None, None, None, :]
    return out


def run(h, adj, w, a_src, a_dst, bias, trace=False, trace_kwargs=None):
    nc = _get_program()
    in_maps = make_in_maps(h, adj, w, a_src, a_dst)
    res = run_bass_kernel_spmd(nc, in_maps, core_ids=list(range(NCORES)),
                               trace=trace, **(trace_kwargs or {}))
    return assemble_output(res.results, bias), res


def kernel(h, adj, w, a_src, a_dst, bias):
    out, _ = run(h, adj, w, a_src, a_dst, bias,
                 trace=bool(int(os.environ.get("GAT_TRACE", "0"))))
    return out


# revision 24
# speedup vs baseline: 1.1302x; 1.1302x over previous
"""Batched multi-head graph attention (GAT) kernel for 8 Trainium2 NeuronCores.

Math (per batch b, head h):
    hp      = h[b] @ w[h]                          # [N, F]
    t       = tanh(hp)
    s       = t @ a_src[h];  d = t @ a_dst[h]      # [N]
    score   = leaky_relu(s_i + d_j, 0.2)
    e       = where(adj>0, exp(score), 0)
    out     = (e / e.sum(-1, keepdim)) @ hp + bias

On-device identity (v2-folded form):
    exp(leaky(z)) = e^{0.2 s_i} * v2_j * max(q_i w_j, 1)
    with q = e^{0.8 s}, w = e^{0.8 d}, v2 = e^{0.2 d}.  The e^{0.2 s_i}
    row factor cancels in softmax.  The v2_j column factor is folded into
    the PE stationary [v2*hp | v2], so the per-element work is ONE
    tensor_scalar (4x DVE mode):  ea = max(q_i * w_j, 1)
    then ONE mask multiply (2x DVE):  da = ea * adjT
    and a PE matmul accumulating numerator and denominator together.

    ACT-offload variant for some (h,jb) blocks: ea_act = relu(q w - 1) on
    the Scalar engine; the missing "+1" is restored by an extra matmul of
    the SAME stationary [v2*hp | v2] against the raw adjT mask (exact).

adj mask trick: adj values 0.0/1.0 fp32; the high uint16 halves bitcast to
fp16 {0, 1.875} -- a constant scale on every surviving term that cancels
in the normalization.  Host passes adj TRANSPOSED as uint16 high halves.

Sharding: 8 cores = 4 batches x 2 head-pairs; each core handles 2 heads for
ALL 2048 query rows against all 2048 keys.  Output is fp16 (host upcasts);
the PSUM spill is scaled by 2^-6 which cancels in the num/den ratio.
"""

import os
from contextlib import ExitStack

import numpy as np

import concourse.bass as bass
import concourse.mybir as mybir
import concourse.tile as tile
from concourse import bacc
from concourse.bass_utils import run_bass_kernel_spmd
from concourse.masks import make_identity

F32 = mybir.dt.float32
F16 = mybir.dt.float16
U16 = mybir.dt.uint16
ALU = mybir.AluOpType
ACTF = mybir.ActivationFunctionType
AX = mybir.AxisListType

B, N, H, F = 4, 2048, 4, 64
NCORES = 8
ROWS = N               # query rows per core (full)
KEYS = N               # keys per core (full)
HEADS_PER = 2          # heads per core
NEG_SLOPE = 0.2
SPILL_SCALE = 2.0 ** -6


def default_assign(jb, h):
    """ea engine for block (jb, head h): "act" -> Scalar relu + PE fixup,
    "dve" -> Vector tensor_scalar max."""
    return "act" if (jb + h) % 2 == 0 else "dve"


# da blocks routed to the Pool engine: measured 4-8us per block there
# (vs 1.2 on Vector) with SBUF port contention stalls -- keep empty
DEFAULT_POOL_DA = frozenset()


def build_program(rows=ROWS, keys=KEYS, heads=HEADS_PER, f=F,
                  assign=default_assign, pool_da=DEFAULT_POOL_DA):
    nc = bacc.Bacc("TRN2", target_bir_lowering=False, debug=False)

    kb = keys // 128           # key blocks
    qb = rows // 128           # query blocks (for q transposes)
    nhalf = rows // 512        # 512-wide output column chunks per head
    fe = f + 1                 # [v2*hp | v2] stationary width

    hbT_d = nc.dram_tensor("hbT", [f, keys], F32, kind="ExternalInput")
    adjt_d = nc.dram_tensor("adjt", [keys, rows], U16, kind="ExternalInput")
    w_d = nc.dram_tensor("wmat", [heads, f, f], F32, kind="ExternalInput")
    # host pre-broadcasts the attention vectors to all 128 partitions
    ap_d = nc.dram_tensor("apairb", [128, heads, 2, f], F32,
                          kind="ExternalInput")
    out_d = nc.dram_tensor("out", [heads, rows, f], F16,
                           kind="ExternalOutput")

    with tile.TileContext(nc) as tc:
        with (
            tc.tile_pool(name="const", bufs=1) as const,
            tc.tile_pool(name="persist", bufs=1) as persist,
            tc.tile_pool(name="stmp", bufs=2) as stmp,
        ):
            id16 = const.tile([128, 128], F16, tag="id16")
            make_identity(nc, id16)
            neg1 = const.tile([128, 1], F32, tag="neg1")
            nc.vector.memset(neg1, -1.0)

            # ---- global loads (small unblockers first) ------------------
            abc32 = persist.tile([128, heads, 2, f], F32, tag="abc32")
            nc.sync.dma_start(out=abc32, in_=ap_d.ap())
            hT32 = persist.tile([f, keys], F32, tag="hT32")
            nc.sync.dma_start(out=hT32, in_=hbT_d.ap())
            w32 = persist.tile([f, heads, f], F32, tag="w32")
            nc.sync.dma_start(out=w32, in_=w_d.ap().rearrange("h f o -> f h o"))

            a16 = persist.tile([128, heads, 2, f], F16, tag="a16")
            nc.vector.tensor_copy(a16, abc32)
            w16 = persist.tile([f, heads, f], F16, tag="w16")
            nc.vector.tensor_copy(w16, w32)
            hT16 = persist.tile([f, keys], F16, tag="hT16")
            # cast in halves so the first hp matmul group starts earlier
            nc.scalar.activation(hT16[:, 0:keys // 2], hT32[:, 0:keys // 2],
                                 ACTF.Copy)
            nc.scalar.activation(hT16[:, keys // 2:], hT32[:, keys // 2:],
                                 ACTF.Copy)

            # ---- prefetch all transposed mask blocks (streamed pool) ----
            # (declared early so DMA runs under the whole setup phase)
            adjp_stack = ExitStack()
            adjp = adjp_stack.enter_context(
                tc.tile_pool(name="adjp", bufs=8))
            adjts = []
            for jb in range(kb):
                adjt_t = adjp.tile([128, rows], U16, tag="adjt",
                                   name=f"adjt{jb}")
                nc.sync.dma_start(
                    out=adjt_t, in_=adjt_d.ap()[jb * 128:(jb + 1) * 128, :])
                adjts.append(adjt_t)

            # ---- per-head setup (head 0's chain completes first) --------
            tanh16 = persist.tile([128, heads, kb, f], F16, tag="tanh16")
            hpt2 = []    # [128, kb, fe] f16 -- [v2*hp | v2] stationaries
            wcol = []    # [128, kb] f32 -- e^{0.8 d}
            qbc = []     # [128, rows] f16 -- e^{0.8 s} broadcast
            with (
                tc.tile_pool(name="psum_hp", bufs=2, space="PSUM") as php,
                tc.tile_pool(name="psum_q", bufs=2, space="PSUM") as pq,
            ):
                for h in range(heads):
                    hpt_h = stmp.tile([128, kb, fe], F16, name=f"hptmp{h}",
                                      tag=f"hptmp{h}")
                    g_hp = min(8, kb)
                    for g in range(kb // g_hp):
                        pp = php.tile([128, g_hp * f], F32, tag="php")
                        for t in range(g_hp):
                            blk = g * g_hp + t
                            nc.tensor.matmul(
                                pp[:, t * f:(t + 1) * f],
                                lhsT=hT16[:, blk * 128:(blk + 1) * 128],
                                rhs=w16[:, h, :], start=True, stop=True)
                        nc.scalar.activation(
                            tanh16[:, h, g * g_hp:(g + 1) * g_hp, :],
                            pp.rearrange("p (t o) -> p t o", o=f),
                            ACTF.Tanh)
                        nc.scalar.activation(
                            hpt_h[:, g * g_hp:(g + 1) * g_hp, 0:f],
                            pp.rearrange("p (t o) -> p t o", o=f),
                            ACTF.Identity)
                    nc.vector.memset(hpt_h[:, :, f:fe], 1.0)

                    # s, d for this head: prod + blockwise reduce (s first:
                    # it unblocks the q chain before d finishes)
                    prod = stmp.tile([128, kb, 2, f], F16, name=f"prod{h}",
                                     tag=f"prod{h}")
                    nc.vector.tensor_tensor(
                        out=prod,
                        in0=tanh16[:, h].unsqueeze(2).broadcast_to(
                            [128, kb, 2, f]),
                        in1=a16[:, h].unsqueeze(1).broadcast_to(
                            [128, kb, 2, f]),
                        op=ALU.mult)
                    sums = stmp.tile([128, kb, 2], F32, name=f"sums{h}",
                                     tag=f"sums{h}")
                    nc.vector.reduce_sum(sums[:, :, 0:1], prod[:, :, 0:1, :],
                                         axis=AX.X)

                    # q chain: exp col (f16), transpose to row, evac,
                    # broadcast (gpsimd runs while DVE does the d reduce)
                    qc16 = stmp.tile([128, kb], F16, name=f"qc{h}",
                                     tag=f"qc{h}")
                    nc.scalar.activation(qc16, sums[:, :, 0], ACTF.Exp,
                                         scale=1.0 - NEG_SLOPE)
                    pq_t = pq.tile([1, rows], F16, tag="pq")
                    for t in range(qb):
                        nc.tensor.transpose(
                            pq_t[:, t * 128:(t + 1) * 128],
                            qc16[:, t:t + 1], id16)
                    qrow = stmp.tile([1, rows], F16, name=f"qrow{h}",
                                     tag=f"qrow{h}")
                    nc.vector.tensor_copy(qrow, pq_t)
                    qb_h = persist.tile([128, rows], F16, tag=f"qb{h}")
                    nc.gpsimd.partition_broadcast(qb_h, qrow)
                    qbc.append(qb_h)

                    # d reduce + w = e^{0.8 d} (scalar for ea), v2 = e^{0.2
                    # d} (f16, folded into the stationary)
                    nc.vector.reduce_sum(sums[:, :, 1:2], prod[:, :, 1:2, :],
                                         axis=AX.X)
                    w_h = persist.tile([128, kb], F32, tag=f"w{h}")
                    nc.scalar.activation(w_h, sums[:, :, 1], ACTF.Exp,
                                         scale=1.0 - NEG_SLOPE)
                    wcol.append(w_h)
                    v2c = stmp.tile([128, kb], F16, name=f"v2c{h}",
                                    tag=f"v2c{h}")
                    nc.scalar.activation(v2c, sums[:, :, 1], ACTF.Exp,
                                         scale=NEG_SLOPE)
                    hpt2_h = persist.tile([128, kb, fe], F16, tag=f"hpt2{h}")
                    nc.gpsimd.tensor_tensor(
                        out=hpt2_h, in0=hpt_h,
                        in1=v2c.unsqueeze(2).broadcast_to([128, kb, fe]),
                        op=ALU.mult)
                    hpt2.append(hpt2_h)

            # ---- main loop: masked weights + fused matmul ---------------
            nacc = heads * nhalf
            acc_sb = persist.tile([fe, nacc, 512], F16, tag="acc_sb")
            accp_stack = ExitStack()
            accp = accp_stack.enter_context(
                tc.tile_pool(name="accp", bufs=1, space="PSUM"))
            accs = {}
            for h in range(heads):
                for half in range(nhalf):
                    i = h * nhalf + half
                    accs[i] = accp.tile([fe, 512], F32, tag=f"acc{i}",
                                        name=f"acc{i}")

            with (
                tc.tile_pool(name="ep", bufs=4) as ep,
                tc.tile_pool(name="dp", bufs=4) as dp,
            ):
                for jb in range(kb):
                    adj16 = adjts[jb].bitcast(F16)
                    last = jb == kb - 1
                    for h in range(heads):
                        is_act = assign(jb, h) == "act"
                        w_s = wcol[h][:, jb:jb + 1]
                        ea = ep.tile([128, rows], F16, tag=f"ea{h}",
                                     name=f"ea{h}_{jb}")
                        if is_act:
                            nc.scalar.activation(
                                ea, qbc[h], ACTF.Relu,
                                bias=neg1, scale=w_s)
                        else:
                            nc.vector.tensor_scalar(
                                out=ea, in0=qbc[h],
                                scalar1=w_s, scalar2=1.0,
                                op0=ALU.mult, op1=ALU.max)
                        da = dp.tile([128, rows], F16, tag=f"da{h}",
                                     name=f"da{h}_{jb}")
                        da_eng = (nc.gpsimd if (jb, h) in pool_da
                                  else nc.vector)
                        da_eng.tensor_tensor(out=da, in0=ea, in1=adj16,
                                             op=ALU.mult)
                        for half in range(nhalf):
                            sl = slice(half * 512, (half + 1) * 512)
                            nc.tensor.matmul(
                                accs[h * nhalf + half],
                                lhsT=hpt2[h][:, jb, :],
                                rhs=da[:, sl],
                                start=(jb == 0),
                                stop=(last and not is_act))
                            if is_act:
                                nc.tensor.matmul(
                                    accs[h * nhalf + half],
                                    lhsT=hpt2[h][:, jb, :],
                                    rhs=adj16[:, sl],
                                    start=False, stop=last)

                # spill accumulators (scaled; scale cancels in num/den);
                # alternate engines so the copies drain in parallel
                for i in range(nacc):
                    if i % 2 == 0:
                        nc.scalar.activation(acc_sb[:, i, :], accs[i],
                                             ACTF.Identity,
                                             scale=SPILL_SCALE)
                    else:
                        nc.vector.tensor_scalar_mul(acc_sb[:, i, :],
                                                    accs[i], SPILL_SCALE)
            accp_stack.close()
            adjp_stack.close()

            # ---- normalize + store (fp16) -------------------------------
            nq = 512 // 128
            with (
                tc.tile_pool(name="ptf", bufs=2, space="PSUM") as ptf,
                tc.tile_pool(name="outp", bufs=4) as outp,
            ):
                for h in range(heads):
                    for half in range(nhalf):
                        i = h * nhalf + half
                        pt = ptf.tile([128, nq, fe + 1], F16, tag="pt")
                        for q in range(nq):
                            nc.tensor.transpose(
                                pt[:, q, 0:fe],
                                acc_sb[:, i, q * 128:(q + 1) * 128],
                                id16[0:fe, 0:fe])
                        rcol = outp.tile([128, nq], F32, tag="rcol")
                        nc.vector.reciprocal(rcol, pt[:, :, f])
                        rc16 = outp.tile([128, nq], F16, tag="rc16")
                        nc.vector.tensor_copy(rc16, rcol)
                        osb = outp.tile([128, nq, f], F16, tag="osb")
                        nc.vector.tensor_tensor(
                            out=osb, in0=pt[:, :, 0:f],
                            in1=rc16.unsqueeze(2).broadcast_to(
                                [128, nq, f]),
                            op=ALU.mult)
                        nc.sync.dma_start(
                            out=out_d.ap()[
                                h, half * 512:(half + 1) * 512, :]
                            .rearrange("(q p) f -> p q f", p=128),
                            in_=osb)
    nc.compile()
    return nc


_PROGRAM_CACHE = {}


def _get_program():
    key = "full"
    if key not in _PROGRAM_CACHE:
        _PROGRAM_CACHE[key] = build_program()
    return _PROGRAM_CACHE[key]


def make_in_maps(h, adj, w, a_src, a_dst):
    """Shard + marshal the full inputs into 8 per-core input maps."""
    h = np.ascontiguousarray(np.asarray(h, dtype=np.float32))
    adj = np.ascontiguousarray(np.asarray(adj, dtype=np.float32))
    w = np.ascontiguousarray(np.asarray(w, dtype=np.float32))
    apairt_all = np.concatenate(
        [np.asarray(a_src)[:, None, :, 0], np.asarray(a_dst)[:, None, :, 0]],
        axis=1).astype(np.float32)  # [H, 2, F]
    in_maps = []
    hbT = {}
    adjt = {}
    for b in range(B):
        hbT[b] = np.ascontiguousarray(h[b].T)  # [F, N]
        adjT = np.ascontiguousarray(adj[b].T)  # [N, N] keys x queries
        adjt[b] = np.ascontiguousarray(
            adjT.view(np.uint16).reshape(N, N, 2)[:, :, 1])
    for c in range(NCORES):
        b, p = c // 2, c % 2
        apair = apairt_all[2 * p:2 * p + 2]  # [2, 2, F]
        apairb = np.ascontiguousarray(
            np.broadcast_to(apair[None], (128,) + apair.shape))
        in_maps.append({
            "hbT": hbT[b],
            "adjt": adjt[b],
            "wmat": np.ascontiguousarray(w[2 * p:2 * p + 2]),
            "apairb": apairb,
        })
    return in_maps


def assemble_output(results, bias):
    """Gather per-core [2, N, F] fp16 results into [B, H, N, F] fp32."""
    out = np.empty((B, H, N, F), dtype=np.float32)
    for c in range(NCORES):
        b, p = c // 2, c % 2
        out[b, 2 * p:2 * p + 2, :, :] = results[c]["out"].astype(np.float32)
    if bias is not None:
        out = out + np.asarray(bias, dtype=np.float32)[None, None, None, :]
    return out


def run(h, adj, w, a_src, a_dst, bias, trace=False, trace_kwargs=None):
    nc = _get_program()
    in_maps = make_in_maps(h, adj, w, a_src, a_dst)
    res = run_bass_kernel_spmd(nc, in_maps, core_ids=list(range(NCORES)),
                               trace=trace, **(trace_kwargs or {}))
    return assemble_output(res.results, bias), res


def kernel(h, adj, w, a_src, a_dst, bias):
    out, _ = run(h, adj, w, a_src, a_dst, bias,
                 trace=bool(int(os.environ.get("GAT_TRACE", "0"))))
    return out


# revision 25
# speedup vs baseline: 1.2026x; 1.0641x over previous
"""Batched multi-head graph attention (GAT) kernel for 8 Trainium2 NeuronCores.

Math (per batch b, head h):
    hp      = h[b] @ w[h]                          # [N, F]
    t       = tanh(hp)
    s       = t @ a_src[h];  d = t @ a_dst[h]      # [N]
    score   = leaky_relu(s_i + d_j, 0.2)
    e       = where(adj>0, exp(score), 0)
    out     = (e / e.sum(-1, keepdim)) @ hp + bias

On-device identity (v2-folded form):
    exp(leaky(z)) = e^{0.2 s_i} * v2_j * max(q_i w_j, 1)
    with q = e^{0.8 s}, w = e^{0.8 d}, v2 = e^{0.2 d}.  The e^{0.2 s_i}
    row factor cancels in softmax.  The v2_j column factor is folded into
    the PE stationary [v2*hp | v2], so the per-element work is ONE
    tensor_scalar (4x DVE mode):  ea = max(q_i * w_j, 1)
    then ONE mask multiply (2x DVE):  da = ea * adjT
    and a PE matmul accumulating numerator and denominator together.

    ACT-offload variant for some (h,jb) blocks: ea_act = relu(q w - 1) on
    the Scalar engine; the missing "+1" is restored by an extra matmul of
    the SAME stationary [v2*hp | v2] against the raw adjT mask (exact).

adj mask trick: adj values 0.0/1.0 fp32; the high uint16 halves bitcast to
fp16 {0, 1.875} -- a constant scale on every surviving term that cancels
in the normalization.  Host passes adj TRANSPOSED as uint16 high halves.

Sharding: 8 cores = 4 batches x 2 head-pairs; each core handles 2 heads for
ALL 2048 query rows against all 2048 keys.  Output is fp16 (host upcasts);
the PSUM spill is scaled by 2^-6 which cancels in the num/den ratio.
"""

import os
from contextlib import ExitStack

import numpy as np

import concourse.bass as bass
import concourse.mybir as mybir
import concourse.tile as tile
from concourse import bacc
from concourse.bass_utils import run_bass_kernel_spmd
from concourse.masks import make_identity

F32 = mybir.dt.float32
F16 = mybir.dt.float16
U16 = mybir.dt.uint16
ALU = mybir.AluOpType
ACTF = mybir.ActivationFunctionType
AX = mybir.AxisListType

B, N, H, F = 4, 2048, 4, 64
NCORES = 8
ROWS = N               # query rows per core (full)
KEYS = N               # keys per core (full)
HEADS_PER = 2          # heads per core
NEG_SLOPE = 0.2
SPILL_SCALE = 2.0 ** -6


def default_assign(jb, h):
    """ea engine for block (jb, head h): "act" -> Scalar relu + PE fixup,
    "dve" -> Vector tensor_scalar max."""
    return "act" if (jb + h) % 2 == 0 else "dve"


# da blocks routed to the Pool engine: measured 4-8us per block there
# (vs 1.2 on Vector) with SBUF port contention stalls -- keep empty
DEFAULT_POOL_DA = frozenset()


def build_program(rows=ROWS, keys=KEYS, heads=HEADS_PER, f=F,
                  assign=default_assign, pool_da=DEFAULT_POOL_DA):
    nc = bacc.Bacc("TRN2", target_bir_lowering=False, debug=False)

    kb = keys // 128           # key blocks
    qb = rows // 128           # query blocks (for q transposes)
    nhalf = rows // 512        # 512-wide output column chunks per head
    fe = f + 1                 # [v2*hp | v2] stationary width

    hbT_d = nc.dram_tensor("hbT", [f, keys], F32, kind="ExternalInput")
    adjt_d = nc.dram_tensor("adjt", [keys, rows], U16, kind="ExternalInput")
    w_d = nc.dram_tensor("wmat", [heads, f, f], F32, kind="ExternalInput")
    # host pre-broadcasts the attention vectors to all 128 partitions
    ap_d = nc.dram_tensor("apairb", [128, heads, 2, f], F32,
                          kind="ExternalInput")
    out_d = nc.dram_tensor("out", [heads, rows, f], F16,
                           kind="ExternalOutput")

    with tile.TileContext(nc) as tc:
        with (
            tc.tile_pool(name="const", bufs=1) as const,
            tc.tile_pool(name="persist", bufs=1) as persist,
            tc.tile_pool(name="stmp", bufs=2) as stmp,
        ):
            id16 = const.tile([128, 128], F16, tag="id16")
            make_identity(nc, id16)
            neg1 = const.tile([128, 1], F32, tag="neg1")
            nc.vector.memset(neg1, -1.0)

            # ---- global loads (small unblockers first) ------------------
            abc32 = persist.tile([128, heads, 2, f], F32, tag="abc32")
            nc.sync.dma_start(out=abc32, in_=ap_d.ap())
            hT32 = persist.tile([f, keys], F32, tag="hT32")
            nc.sync.dma_start(out=hT32, in_=hbT_d.ap())
            w32 = persist.tile([f, heads, f], F32, tag="w32")
            nc.sync.dma_start(out=w32, in_=w_d.ap().rearrange("h f o -> f h o"))

            a16 = persist.tile([128, heads, 2, f], F16, tag="a16")
            nc.vector.tensor_copy(a16, abc32)
            w16 = persist.tile([f, heads, f], F16, tag="w16")
            nc.vector.tensor_copy(w16, w32)
            hT16 = persist.tile([f, keys], F16, tag="hT16")
            # cast in halves so the first hp matmul group starts earlier
            nc.scalar.activation(hT16[:, 0:keys // 2], hT32[:, 0:keys // 2],
                                 ACTF.Copy)
            nc.scalar.activation(hT16[:, keys // 2:], hT32[:, keys // 2:],
                                 ACTF.Copy)

            # ---- prefetch all transposed mask blocks (streamed pool) ----
            # (declared early so DMA runs under the whole setup phase)
            adjp_stack = ExitStack()
            adjp = adjp_stack.enter_context(
                tc.tile_pool(name="adjp", bufs=8))
            adjts = []
            for jb in range(kb):
                adjt_t = adjp.tile([128, rows], U16, tag="adjt",
                                   name=f"adjt{jb}")
                nc.sync.dma_start(
                    out=adjt_t, in_=adjt_d.ap()[jb * 128:(jb + 1) * 128, :])
                adjts.append(adjt_t)

            # ---- per-head setup (head 0's chain completes first) --------
            tanh16 = persist.tile([128, heads, kb, f], F16, tag="tanh16")
            hpt2 = []    # [128, kb, fe] f16 -- [v2*hp | v2] stationaries
            wcol = []    # [128, kb] f32 -- e^{0.8 d}
            qbc = []     # [128, rows] f16 -- e^{0.8 s} broadcast
            with (
                tc.tile_pool(name="psum_hp", bufs=2, space="PSUM") as php,
                tc.tile_pool(name="psum_q", bufs=2, space="PSUM") as pq,
            ):
                for h in range(heads):
                    hpt_h = stmp.tile([128, kb, fe], F16, name=f"hptmp{h}",
                                      tag=f"hptmp{h}")
                    g_hp = min(8, kb)
                    for g in range(kb // g_hp):
                        pp = php.tile([128, g_hp * f], F32, tag="php")
                        for t in range(g_hp):
                            blk = g * g_hp + t
                            nc.tensor.matmul(
                                pp[:, t * f:(t + 1) * f],
                                lhsT=hT16[:, blk * 128:(blk + 1) * 128],
                                rhs=w16[:, h, :], start=True, stop=True)
                        nc.scalar.activation(
                            tanh16[:, h, g * g_hp:(g + 1) * g_hp, :],
                            pp.rearrange("p (t o) -> p t o", o=f),
                            ACTF.Tanh)
                        nc.scalar.activation(
                            hpt_h[:, g * g_hp:(g + 1) * g_hp, 0:f],
                            pp.rearrange("p (t o) -> p t o", o=f),
                            ACTF.Identity)
                    nc.vector.memset(hpt_h[:, :, f:fe], 1.0)

                    # s, d for this head: prod + blockwise reduce (s first:
                    # it unblocks the q chain before d finishes)
                    prod = stmp.tile([128, kb, 2, f], F16, name=f"prod{h}",
                                     tag=f"prod{h}")
                    nc.vector.tensor_tensor(
                        out=prod,
                        in0=tanh16[:, h].unsqueeze(2).broadcast_to(
                            [128, kb, 2, f]),
                        in1=a16[:, h].unsqueeze(1).broadcast_to(
                            [128, kb, 2, f]),
                        op=ALU.mult)
                    sums = stmp.tile([128, kb, 2], F32, name=f"sums{h}",
                                     tag=f"sums{h}")
                    nc.vector.reduce_sum(sums[:, :, 0:1], prod[:, :, 0:1, :],
                                         axis=AX.X)

                    # q chain: exp col (f16), transpose to row, evac,
                    # broadcast (gpsimd runs while DVE does the d reduce)
                    qc16 = stmp.tile([128, kb], F16, name=f"qc{h}",
                                     tag=f"qc{h}")
                    nc.scalar.activation(qc16, sums[:, :, 0], ACTF.Exp,
                                         scale=1.0 - NEG_SLOPE)
                    pq_t = pq.tile([1, rows], F16, tag="pq")
                    for t in range(qb):
                        nc.tensor.transpose(
                            pq_t[:, t * 128:(t + 1) * 128],
                            qc16[:, t:t + 1], id16)
                    qrow = stmp.tile([1, rows], F16, name=f"qrow{h}",
                                     tag=f"qrow{h}")
                    nc.vector.tensor_copy(qrow, pq_t)
                    qb_h = persist.tile([128, rows], F16, tag=f"qb{h}")
                    nc.gpsimd.partition_broadcast(qb_h, qrow)
                    qbc.append(qb_h)

                    # d reduce + w = e^{0.8 d} (scalar for ea), v2 = e^{0.2
                    # d} (f16, folded into the stationary)
                    nc.vector.reduce_sum(sums[:, :, 1:2], prod[:, :, 1:2, :],
                                         axis=AX.X)
                    w_h = persist.tile([128, kb], F32, tag=f"w{h}")
                    nc.scalar.activation(w_h, sums[:, :, 1], ACTF.Exp,
                                         scale=1.0 - NEG_SLOPE)
                    wcol.append(w_h)
                    v2c = stmp.tile([128, kb], F16, name=f"v2c{h}",
                                    tag=f"v2c{h}")
                    nc.scalar.activation(v2c, sums[:, :, 1], ACTF.Exp,
                                         scale=NEG_SLOPE)
                    hpt2_h = persist.tile([128, kb, fe], F16, tag=f"hpt2{h}")
                    nc.vector.tensor_tensor(
                        out=hpt2_h, in0=hpt_h,
                        in1=v2c.unsqueeze(2).broadcast_to([128, kb, fe]),
                        op=ALU.mult)
                    hpt2.append(hpt2_h)

            # ---- main loop: masked weights + fused matmul ---------------
            nacc = heads * nhalf
            acc_sb = persist.tile([fe, nacc, 512], F16, tag="acc_sb")
            accp_stack = ExitStack()
            accp = accp_stack.enter_context(
                tc.tile_pool(name="accp", bufs=1, space="PSUM"))
            accs = {}
            for h in range(heads):
                for half in range(nhalf):
                    i = h * nhalf + half
                    accs[i] = accp.tile([fe, 512], F32, tag=f"acc{i}",
                                        name=f"acc{i}")

            with (
                tc.tile_pool(name="ep", bufs=4) as ep,
                tc.tile_pool(name="dp", bufs=4) as dp,
            ):
                for jb in range(kb):
                    adj16 = adjts[jb].bitcast(F16)
                    last = jb == kb - 1
                    for h in range(heads):
                        is_act = assign(jb, h) == "act"
                        w_s = wcol[h][:, jb:jb + 1]
                        ea = ep.tile([128, rows], F16, tag=f"ea{h}",
                                     name=f"ea{h}_{jb}")
                        if is_act:
                            nc.scalar.activation(
                                ea, qbc[h], ACTF.Relu,
                                bias=neg1, scale=w_s)
                        else:
                            nc.vector.tensor_scalar(
                                out=ea, in0=qbc[h],
                                scalar1=w_s, scalar2=1.0,
                                op0=ALU.mult, op1=ALU.max)
                        da = dp.tile([128, rows], F16, tag=f"da{h}",
                                     name=f"da{h}_{jb}")
                        da_eng = (nc.gpsimd if (jb, h) in pool_da
                                  else nc.vector)
                        da_eng.tensor_tensor(out=da, in0=ea, in1=adj16,
                                             op=ALU.mult)
                        for half in range(nhalf):
                            sl = slice(half * 512, (half + 1) * 512)
                            nc.tensor.matmul(
                                accs[h * nhalf + half],
                                lhsT=hpt2[h][:, jb, :],
                                rhs=da[:, sl],
                                start=(jb == 0),
                                stop=(last and not is_act))
                            if is_act:
                                nc.tensor.matmul(
                                    accs[h * nhalf + half],
                                    lhsT=hpt2[h][:, jb, :],
                                    rhs=adj16[:, sl],
                                    start=False, stop=last)

                # spill accumulators (scaled; scale cancels in num/den);
                # alternate engines so the copies drain in parallel
                for i in range(nacc):
                    if i % 2 == 0:
                        nc.scalar.activation(acc_sb[:, i, :], accs[i],
                                             ACTF.Identity,
                                             scale=SPILL_SCALE)
                    else:
                        nc.vector.tensor_scalar_mul(acc_sb[:, i, :],
                                                    accs[i], SPILL_SCALE)
            accp_stack.close()
            adjp_stack.close()

            # ---- normalize + store (fp16) -------------------------------
            nq = 512 // 128
            with (
                tc.tile_pool(name="ptf", bufs=2, space="PSUM") as ptf,
                tc.tile_pool(name="outp", bufs=4) as outp,
            ):
                for h in range(heads):
                    for half in range(nhalf):
                        i = h * nhalf + half
                        pt = ptf.tile([128, nq, fe + 1], F16, tag="pt")
                        for q in range(nq):
                            nc.tensor.transpose(
                                pt[:, q, 0:fe],
                                acc_sb[:, i, q * 128:(q + 1) * 128],
                                id16[0:fe, 0:fe])
                        rcol = outp.tile([128, nq], F32, tag="rcol")
                        nc.vector.reciprocal(rcol, pt[:, :, f])
                        rc16 = outp.tile([128, nq], F16, tag="rc16")
                        nc.vector.tensor_copy(rc16, rcol)
                        osb = outp.tile([128, nq, f], F16, tag="osb")
                        nc.vector.tensor_tensor(
                            out=osb, in0=pt[:, :, 0:f],
                            in1=rc16.unsqueeze(2).broadcast_to(
                                [128, nq, f]),
                            op=ALU.mult)
                        nc.sync.dma_start(
                            out=out_d.ap()[
                                h, half * 512:(half + 1) * 512, :]
                            .rearrange("(q p) f -> p q f", p=128),
                            in_=osb)
    nc.compile()
    return nc


_PROGRAM_CACHE = {}


def _get_program():
    key = "full"
    if key not in _PROGRAM_CACHE:
        _PROGRAM_CACHE[key] = build_program()
    return _PROGRAM_CACHE[key]


def make_in_maps(h, adj, w, a_src, a_dst):
    """Shard + marshal the full inputs into 8 per-core input maps."""
    h = np.ascontiguousarray(np.asarray(h, dtype=np.float32))
    adj = np.ascontiguousarray(np.asarray(adj, dtype=np.float32))
    w = np.ascontiguousarray(np.asarray(w, dtype=np.float32))
    apairt_all = np.concatenate(
        [np.asarray(a_src)[:, None, :, 0], np.asarray(a_dst)[:, None, :, 0]],
        axis=1).astype(np.float32)  # [H, 2, F]
    in_maps = []
    hbT = {}
    adjt = {}
    for b in range(B):
        hbT[b] = np.ascontiguousarray(h[b].T)  # [F, N]
        adjT = np.ascontiguousarray(adj[b].T)  # [N, N] keys x queries
        adjt[b] = np.ascontiguousarray(
            adjT.view(np.uint16).reshape(N, N, 2)[:, :, 1])
    for c in range(NCORES):
        b, p = c // 2, c % 2
        apair = apairt_all[2 * p:2 * p + 2]  # [2, 2, F]
        apairb = np.ascontiguousarray(
            np.broadcast_to(apair[None], (128,) + apair.shape))
        in_maps.append({
            "hbT": hbT[b],
            "adjt": adjt[b],
            "wmat": np.ascontiguousarray(w[2 * p:2 * p + 2]),
            "apairb": apairb,
        })
    return in_maps


def assemble_output(results, bias):
    """Gather per-core [2, N, F] fp16 results into [B, H, N, F] fp32."""
    out = np.empty((B, H, N, F), dtype=np.float32)
    for c in range(NCORES):
        b, p = c // 2, c % 2
        out[b, 2 * p:2 * p + 2, :, :] = results[c]["out"].astype(np.float32)
    if bias is not None:
        out = out + np.asarray(bias, dtype=np.float32)[None, None, None, :]
    return out


def run(h, adj, w, a_src, a_dst, bias, trace=False, trace_kwargs=None):
    nc = _get_program()
    in_maps = make_in_maps(h, adj, w, a_src, a_dst)
    res = run_bass_kernel_spmd(nc, in_maps, core_ids=list(range(NCORES)),
                               trace=trace, **(trace_kwargs or {}))
    return assemble_output(res.results, bias), res


def kernel(h, adj, w, a_src, a_dst, bias):
    out, _ = run(h, adj, w, a_src, a_dst, bias,
                 trace=bool(int(os.environ.get("GAT_TRACE", "0"))))
    return out


# revision 27
# speedup vs baseline: 1.2855x; 1.0689x over previous
"""Batched multi-head graph attention (GAT) kernel for 8 Trainium2 NeuronCores.

Math (per batch b, head h):
    hp      = h[b] @ w[h]                          # [N, F]
    t       = tanh(hp)
    s       = t @ a_src[h];  d = t @ a_dst[h]      # [N]
    score   = leaky_relu(s_i + d_j, 0.2)
    e       = where(adj>0, exp(score), 0)
    out     = (e / e.sum(-1, keepdim)) @ hp + bias

On-device identity (v2-folded form):
    exp(leaky(z)) = e^{0.2 s_i} * v2_j * max(q_i w_j, 1)
    with q = e^{0.8 s}, w = e^{0.8 d}, v2 = e^{0.2 d}.  The e^{0.2 s_i}
    row factor cancels in softmax.  The v2_j column factor is folded into
    the PE stationary [v2*hp | v2], so the per-element work is ONE
    tensor_scalar (4x DVE mode):  ea = max(q_i * w_j, 1)
    then ONE mask multiply (2x DVE):  da = ea * adjT
    and a PE matmul accumulating numerator and denominator together.

    ACT-offload variant for some (h,jb) blocks: ea_act = relu(q w - 1) on
    the Scalar engine; the missing "+1" is restored by an extra matmul of
    the SAME stationary [v2*hp | v2] against the raw adjT mask (exact).

adj mask trick: adj values 0.0/1.0 fp32; the high uint16 halves bitcast to
fp16 {0, 1.875} -- a constant scale on every surviving term that cancels
in the normalization.  Host passes adj TRANSPOSED as uint16 high halves.

Sharding: 8 cores = 4 batches x 2 head-pairs; each core handles 2 heads for
ALL 2048 query rows against all 2048 keys.  Output is fp16 (host upcasts);
the PSUM spill is scaled by 2^-6 which cancels in the num/den ratio.
"""

import os
from contextlib import ExitStack

import numpy as np

import concourse.bass as bass
import concourse.mybir as mybir
import concourse.tile as tile
from concourse import bacc
from concourse.bass_utils import run_bass_kernel_spmd
from concourse.masks import make_identity

F32 = mybir.dt.float32
F16 = mybir.dt.float16
U16 = mybir.dt.uint16
ALU = mybir.AluOpType
ACTF = mybir.ActivationFunctionType
AX = mybir.AxisListType

B, N, H, F = 4, 2048, 4, 64
NCORES = 8
ROWS = N               # query rows per core (full)
KEYS = N               # keys per core (full)
HEADS_PER = 2          # heads per core
NEG_SLOPE = 0.2
SPILL_SCALE = 2.0 ** -6


def default_assign(jb, h):
    """ea engine for block (jb, head h): "act" -> Scalar relu + PE fixup,
    "dve" -> Vector tensor_scalar max."""
    return "act" if (jb + h) % 2 == 0 else "dve"


# da blocks routed to the Pool engine: measured 4-8us per block there
# (vs 1.2 on Vector) with SBUF port contention stalls -- keep empty
DEFAULT_POOL_DA = frozenset()


def build_program(rows=ROWS, keys=KEYS, heads=HEADS_PER, f=F,
                  assign=default_assign, pool_da=DEFAULT_POOL_DA):
    nc = bacc.Bacc("TRN2", target_bir_lowering=False, debug=False)

    kb = keys // 128           # key blocks
    qb = rows // 128           # query blocks (for q transposes)
    nhalf = rows // 512        # 512-wide output column chunks per head
    fe = f + 1                 # [v2*hp | v2] stationary width

    hbT_d = nc.dram_tensor("hbT", [f, keys], F16, kind="ExternalInput")
    adjt_d = nc.dram_tensor("adjt", [keys, rows], U16, kind="ExternalInput")
    w_d = nc.dram_tensor("wmat", [heads, f, f], F16, kind="ExternalInput")
    # host pre-broadcasts the attention vectors to all 128 partitions
    ap_d = nc.dram_tensor("apairb", [128, heads, 2, f], F16,
                          kind="ExternalInput")
    qs_d = nc.dram_tensor("qscratch", [2, rows], F16, kind="Internal")
    out_d = nc.dram_tensor("out", [heads, rows, f], F16,
                           kind="ExternalOutput")

    with tile.TileContext(nc) as tc:
        with (
            tc.tile_pool(name="const", bufs=1) as const,
            tc.tile_pool(name="persist", bufs=1) as persist,
            tc.tile_pool(name="stmp", bufs=2) as stmp,
        ):
            id16 = const.tile([128, 128], F16, tag="id16")
            make_identity(nc, id16)
            neg1 = const.tile([128, 1], F32, tag="neg1")
            nc.vector.memset(neg1, -1.0)

            # ---- global loads (small unblockers first, all f16) ---------
            a16 = persist.tile([128, heads, 2, f], F16, tag="a16")
            nc.sync.dma_start(out=a16, in_=ap_d.ap())
            w16 = persist.tile([f, heads, f], F16, tag="w16")
            nc.sync.dma_start(out=w16, in_=w_d.ap().rearrange("h f o -> f h o"))
            hT16 = persist.tile([f, keys], F16, tag="hT16")
            nc.sync.dma_start(out=hT16, in_=hbT_d.ap())

            # ---- prefetch all transposed mask blocks (streamed pool) ----
            # (declared early so DMA runs under the whole setup phase)
            adjp_stack = ExitStack()
            adjp = adjp_stack.enter_context(
                tc.tile_pool(name="adjp", bufs=8))
            adjts = []
            for jb in range(kb):
                adjt_t = adjp.tile([128, rows], U16, tag="adjt",
                                   name=f"adjt{jb}")
                nc.sync.dma_start(
                    out=adjt_t, in_=adjt_d.ap()[jb * 128:(jb + 1) * 128, :])
                adjts.append(adjt_t)

            # ---- per-head setup (head 0's chain completes first) --------
            tanh16 = persist.tile([128, heads, kb, f], F16, tag="tanh16")
            hpt2 = []    # [128, kb, fe] f16 -- [v2*hp | v2] stationaries
            wcol = []    # [128, kb] f32 -- e^{0.8 d}
            qbc = []     # [128, rows] f16 -- e^{0.8 s} broadcast
            with (
                tc.tile_pool(name="psum_hp", bufs=2, space="PSUM") as php,
                tc.tile_pool(name="psum_q", bufs=2, space="PSUM") as pq,
            ):
                for h in range(heads):
                    hpt_h = stmp.tile([128, kb, fe], F16, name=f"hptmp{h}",
                                      tag=f"hptmp{h}")
                    g_hp = min(8, kb)
                    for g in range(kb // g_hp):
                        pp = php.tile([128, g_hp * f], F32, tag="php")
                        for t in range(g_hp):
                            blk = g * g_hp + t
                            nc.tensor.matmul(
                                pp[:, t * f:(t + 1) * f],
                                lhsT=hT16[:, blk * 128:(blk + 1) * 128],
                                rhs=w16[:, h, :], start=True, stop=True)
                        nc.scalar.activation(
                            tanh16[:, h, g * g_hp:(g + 1) * g_hp, :],
                            pp.rearrange("p (t o) -> p t o", o=f),
                            ACTF.Tanh)
                        nc.scalar.activation(
                            hpt_h[:, g * g_hp:(g + 1) * g_hp, 0:f],
                            pp.rearrange("p (t o) -> p t o", o=f),
                            ACTF.Identity)
                    nc.vector.memset(hpt_h[:, :, f:fe], 1.0)

                    # s, d for this head: prod + blockwise reduce (s first:
                    # it unblocks the q chain before d finishes)
                    prod = stmp.tile([128, kb, 2, f], F16, name=f"prod{h}",
                                     tag=f"prod{h}")
                    nc.vector.tensor_tensor(
                        out=prod,
                        in0=tanh16[:, h].unsqueeze(2).broadcast_to(
                            [128, kb, 2, f]),
                        in1=a16[:, h].unsqueeze(1).broadcast_to(
                            [128, kb, 2, f]),
                        op=ALU.mult)
                    sums = stmp.tile([128, kb, 2], F32, name=f"sums{h}",
                                     tag=f"sums{h}")
                    nc.vector.reduce_sum(sums[:, :, 0:1], prod[:, :, 0:1, :],
                                         axis=AX.X)

                    # q chain: exp col (f16), transpose to row, evac,
                    # broadcast (gpsimd runs while DVE does the d reduce)
                    qc16 = stmp.tile([128, kb], F16, name=f"qc{h}",
                                     tag=f"qc{h}")
                    nc.scalar.activation(qc16, sums[:, :, 0], ACTF.Exp,
                                         scale=1.0 - NEG_SLOPE)
                    pq_t = pq.tile([1, rows], F16, tag="pq")
                    for t in range(qb):
                        nc.tensor.transpose(
                            pq_t[:, t * 128:(t + 1) * 128],
                            qc16[:, t:t + 1], id16)
                    qrow = stmp.tile([1, rows], F16, name=f"qrow{h}",
                                     tag=f"qrow{h}")
                    nc.vector.tensor_copy(qrow, pq_t)
                    nc.sync.dma_start(out=qs_d.ap()[h:h + 1, :], in_=qrow)
                    qb_h = persist.tile([128, rows], F16, tag=f"qb{h}")
                    nc.sync.dma_start(
                        out=qb_h,
                        in_=qs_d.ap()[h:h + 1, :].broadcast_to([128, rows]))
                    qbc.append(qb_h)

                    # d reduce + w = e^{0.8 d} (scalar for ea), v2 = e^{0.2
                    # d} (f16, folded into the stationary)
                    nc.vector.reduce_sum(sums[:, :, 1:2], prod[:, :, 1:2, :],
                                         axis=AX.X)
                    w_h = persist.tile([128, kb], F32, tag=f"w{h}")
                    nc.scalar.activation(w_h, sums[:, :, 1], ACTF.Exp,
                                         scale=1.0 - NEG_SLOPE)
                    wcol.append(w_h)
                    v2c = stmp.tile([128, kb], F16, name=f"v2c{h}",
                                    tag=f"v2c{h}")
                    nc.scalar.activation(v2c, sums[:, :, 1], ACTF.Exp,
                                         scale=NEG_SLOPE)
                    hpt2_h = persist.tile([128, kb, fe], F16, tag=f"hpt2{h}")
                    nc.vector.tensor_tensor(
                        out=hpt2_h, in0=hpt_h,
                        in1=v2c.unsqueeze(2).broadcast_to([128, kb, fe]),
                        op=ALU.mult)
                    hpt2.append(hpt2_h)

            # ---- main loop: masked weights + fused matmul ---------------
            nacc = heads * nhalf
            acc_sb = persist.tile([fe, nacc, 512], F16, tag="acc_sb")
            accp_stack = ExitStack()
            accp = accp_stack.enter_context(
                tc.tile_pool(name="accp", bufs=1, space="PSUM"))
            accs = {}
            for h in range(heads):
                for half in range(nhalf):
                    i = h * nhalf + half
                    accs[i] = accp.tile([fe, 512], F32, tag=f"acc{i}",
                                        name=f"acc{i}")

            with (
                tc.tile_pool(name="ep", bufs=4) as ep,
                tc.tile_pool(name="dp", bufs=4) as dp,
            ):
                for jb in range(kb):
                    adj16 = adjts[jb].bitcast(F16)
                    last = jb == kb - 1
                    for h in range(heads):
                        is_act = assign(jb, h) == "act"
                        w_s = wcol[h][:, jb:jb + 1]
                        ea = ep.tile([128, rows], F16, tag=f"ea{h}",
                                     name=f"ea{h}_{jb}")
                        if is_act:
                            nc.scalar.activation(
                                ea, qbc[h], ACTF.Relu,
                                bias=neg1, scale=w_s)
                        else:
                            nc.vector.tensor_scalar(
                                out=ea, in0=qbc[h],
                                scalar1=w_s, scalar2=1.0,
                                op0=ALU.mult, op1=ALU.max)
                        da = dp.tile([128, rows], F16, tag=f"da{h}",
                                     name=f"da{h}_{jb}")
                        da_eng = (nc.gpsimd if (jb, h) in pool_da
                                  else nc.vector)
                        da_eng.tensor_tensor(out=da, in0=ea, in1=adj16,
                                             op=ALU.mult)
                        for half in range(nhalf):
                            sl = slice(half * 512, (half + 1) * 512)
                            nc.tensor.matmul(
                                accs[h * nhalf + half],
                                lhsT=hpt2[h][:, jb, :],
                                rhs=da[:, sl],
                                start=(jb == 0),
                                stop=(last and not is_act))
                            if is_act:
                                nc.tensor.matmul(
                                    accs[h * nhalf + half],
                                    lhsT=hpt2[h][:, jb, :],
                                    rhs=adj16[:, sl],
                                    start=False, stop=last)

                # spill accumulators (scaled; scale cancels in num/den);
                # alternate engines so the copies drain in parallel
                for i in range(nacc):
                    if i % 2 == 0:
                        nc.scalar.activation(acc_sb[:, i, :], accs[i],
                                             ACTF.Identity,
                                             scale=SPILL_SCALE)
                    else:
                        nc.vector.tensor_scalar_mul(acc_sb[:, i, :],
                                                    accs[i], SPILL_SCALE)
            accp_stack.close()
            adjp_stack.close()

            # ---- normalize + store (fp16) -------------------------------
            nq = 512 // 128
            with (
                tc.tile_pool(name="ptf", bufs=2, space="PSUM") as ptf,
                tc.tile_pool(name="outp", bufs=4) as outp,
            ):
                for h in range(heads):
                    for half in range(nhalf):
                        i = h * nhalf + half
                        pt = ptf.tile([128, nq, fe + 1], F16, tag="pt")
                        for q in range(nq):
                            nc.tensor.transpose(
                                pt[:, q, 0:fe],
                                acc_sb[:, i, q * 128:(q + 1) * 128],
                                id16[0:fe, 0:fe])
                        rcol = outp.tile([128, nq], F32, tag="rcol")
                        nc.vector.reciprocal(rcol, pt[:, :, f])
                        rc16 = outp.tile([128, nq], F16, tag="rc16")
                        nc.vector.tensor_copy(rc16, rcol)
                        osb = outp.tile([128, nq, f], F16, tag="osb")
                        nc.vector.tensor_tensor(
                            out=osb, in0=pt[:, :, 0:f],
                            in1=rc16.unsqueeze(2).broadcast_to(
                                [128, nq, f]),
                            op=ALU.mult)
                        nc.sync.dma_start(
                            out=out_d.ap()[
                                h, half * 512:(half + 1) * 512, :]
                            .rearrange("(q p) f -> p q f", p=128),
                            in_=osb)
    nc.compile()
    return nc


_PROGRAM_CACHE = {}


def _get_program():
    key = "full"
    if key not in _PROGRAM_CACHE:
        _PROGRAM_CACHE[key] = build_program()
    return _PROGRAM_CACHE[key]


def make_in_maps(h, adj, w, a_src, a_dst):
    """Shard + marshal the full inputs into 8 per-core input maps."""
    h = np.ascontiguousarray(np.asarray(h, dtype=np.float32))
    adj = np.ascontiguousarray(np.asarray(adj, dtype=np.float32))
    w = np.ascontiguousarray(np.asarray(w, dtype=np.float32))
    apairt_all = np.concatenate(
        [np.asarray(a_src)[:, None, :, 0], np.asarray(a_dst)[:, None, :, 0]],
        axis=1).astype(np.float32)  # [H, 2, F]
    in_maps = []
    hbT = {}
    adjt = {}
    for b in range(B):
        hbT[b] = np.ascontiguousarray(h[b].T.astype(np.float16))  # [F, N]
        adjT = np.ascontiguousarray(adj[b].T)  # [N, N] keys x queries
        adjt[b] = np.ascontiguousarray(
            adjT.view(np.uint16).reshape(N, N, 2)[:, :, 1])
    for c in range(NCORES):
        b, p = c // 2, c % 2
        apair = apairt_all[2 * p:2 * p + 2].astype(np.float16)  # [2, 2, F]
        apairb = np.ascontiguousarray(
            np.broadcast_to(apair[None], (128,) + apair.shape))
        in_maps.append({
            "hbT": hbT[b],
            "adjt": adjt[b],
            "wmat": np.ascontiguousarray(
                w[2 * p:2 * p + 2].astype(np.float16)),
            "apairb": apairb,
        })
    return in_maps


def assemble_output(results, bias):
    """Gather per-core [2, N, F] fp16 results into [B, H, N, F] fp32."""
    out = np.empty((B, H, N, F), dtype=np.float32)
    for c in range(NCORES):
        b, p = c // 2, c % 2
        out[b, 2 * p:2 * p + 2, :, :] = results[c]["out"].astype(np.float32)
    if bias is not None:
        out = out + np.asarray(bias, dtype=np.float32)[None, None, None, :]
    return out


def run(h, adj, w, a_src, a_dst, bias, trace=False, trace_kwargs=None):
    nc = _get_program()
    in_maps = make_in_maps(h, adj, w, a_src, a_dst)
    res = run_bass_kernel_spmd(nc, in_maps, core_ids=list(range(NCORES)),
                               trace=trace, **(trace_kwargs or {}))
    return assemble_output(res.results, bias), res


def kernel(h, adj, w, a_src, a_dst, bias):
    out, _ = run(h, adj, w, a_src, a_dst, bias,
                 trace=bool(int(os.environ.get("GAT_TRACE", "0"))))
    return out
